# revision 39
# baseline (speedup 1.0000x reference)
"""Trainium2 Bass kernel for nn_BD dense MLP (block-diagonal hidden layers).

Network: x[B,64] -> relu(x@W_in)[B,32] -> 4x relu(h@(mask*W_h))[B,32]
         -> h@(mask*W_out)[B,24]

Strategy (pure data parallel over 8 cores, B=1048576, R=131072 rows/core):
 - x loaded batch-major contiguously; DVE 32x32 block-transpose flips each
   [32 batch x 32 feat] block to feature-major. The resulting batch
   permutation is undone by the output DMA access pattern.
 - All matmuls run feature-major: weights stationary (lhsT), activations
   moving (rhs, N=512). 4 batch chunks sit at partition groups 0..3 and use
   diagonal PE tile positions (rows 32c, cols 32c) so 4 matmuls stream
   concurrently in different 32x32 sub-arrays.
 - ReLU fused into the PSUM->SBUF move on ScalarE/VectorE at full 128
   partitions.
 - Output written padded [R,32]; host strips to 24 cols.
"""

import sys

import numpy as np

if "/opt/trn_rl_repo" not in sys.path:
    sys.path.insert(0, "/opt/trn_rl_repo")

N_CORES = 8
B_FULL = 1048576
R = B_FULL // N_CORES  # rows per core
SLAB = 8192  # rows per pipeline iteration
F32 = None  # set after import


def build_nc(rows=R, act_split=(True, True, True, True, False)):
    """Build the single-core SPMD Bass graph.

    act_split[l]: True -> relu on ScalarE, False -> relu on VectorE.
    """
    import concourse.bass as bass  # noqa: F401
    import concourse.mybir as mybir
    from concourse import bacc, tile

    f32 = mybir.dt.float32
    bf16 = mybir.dt.bfloat16
    nc = bacc.Bacc(None)

    x_ext = nc.declare_dram_parameter("x", [rows, 64], bf16, isOutput=False)
    # 7 block-diagonal 128x128 stationaries: L1 fb0, L1 fb1, L2..L5, L6
    wbd_ext = nc.declare_dram_parameter("wbd", [128, 896], bf16, isOutput=False)
    out_ext = nc.declare_dram_parameter("out", [rows, 32], f32, isOutput=True)

    n_slabs = rows // SLAB
    # x row r = s*8192 + p*64 + n  (p = SBUF partition, n = 0..63)
    x_r = x_ext.rearrange("(s p n) f -> s p (n f)", p=128, n=64)
    # out row r = s*8192 + pg*2048 + b*64 + n ; partition = 32*pg + b
    o_r = out_ext.rearrange("(s pg b n) c -> s (pg b) (n c)", pg=4, b=32, n=64)

    Relu = mybir.ActivationFunctionType.Relu

    NILV = 4  # slabs processed in interleaved groups
    assert n_slabs % NILV == 0 or n_slabs < NILV

    with tile.TileContext(nc) as tc:
        with (
            tc.tile_pool(name="const", bufs=1) as cpool,
            tc.tile_pool(name="xin", bufs=4) as xpool,
            tc.tile_pool(name="xt", bufs=3) as xtpool,
            tc.tile_pool(name="h", bufs=12) as hpool,
            tc.tile_pool(name="ps", bufs=2, space="PSUM") as pspool,
            tc.tile_pool(name="ot", bufs=3) as otpool,
        ):
            wbd = cpool.tile([128, 896], bf16, tag="wbd")
            nc.sync.dma_start(wbd[:, :], wbd_ext[:, :])

            def wsl(i):
                return wbd[:, 128 * i : 128 * i + 128]

            def relu(out_t, in_t, on_act):
                if on_act:
                    nc.scalar.activation(out_t, in_t, Relu)
                else:
                    nc.vector.tensor_scalar_max(out_t, in_t, 0.0)

            # Fully skewed software pipeline: step t advances slab t-k
            # through stage k. Stages: 0 load, 1 xT, 2 L1+relu1,
            # 3..6 L2..L5+relu, 7 L6+oT+store.
            st = [dict() for _ in range(n_slabs)]

            def ok(i):
                return 0 <= i < n_slabs

            for t in range(n_slabs + 8):
                if ok(t):
                    x_sb = xpool.tile([128, 4096], bf16, tag="x")
                    nc.sync.dma_start(x_sb[:, :], x_r[t])
                    st[t]["x"] = x_sb

                if ok(t - 1):
                    s = t - 1
                    xt = xtpool.tile([128, 4096], bf16, tag="xt")
                    nc.vector.transpose(xt[:, :], st[s]["x"][:, :])
                    st[s]["xt"] = xt[:, :].rearrange(
                        "p (n fb b) -> p n fb b", fb=2, b=32
                    )

                if ok(t - 2):
                    s = t - 2
                    ps = pspool.tile([128, 2048], f32, tag="ps")
                    for fb in range(2):  # fb outer: 4 MMs share one LDWEIGHTS
                        for hh in range(4):
                            nc.tensor.matmul(
                                ps[:, 512 * hh : 512 * hh + 512],
                                lhsT=wsl(fb),
                                rhs=st[s]["xt"][:, 16 * hh : 16 * hh + 16, fb, :],
                                start=(fb == 0),
                                stop=(fb == 1),
                            )
                    h = hpool.tile([128, 2048], bf16, tag="h")
                    relu(h[:, :], ps[:, :], True)
                    st[s]["h"] = h

                for l in range(4):
                    s = t - 3 - l
                    if ok(s):
                        ps = pspool.tile([128, 2048], f32, tag="ps")
                        for hh in range(4):
                            nc.tensor.matmul(
                                ps[:, 512 * hh : 512 * hh + 512],
                                lhsT=wsl(2 + l),
                                rhs=st[s]["h"][:, 512 * hh : 512 * hh + 512],
                                start=True,
                                stop=True,
                            )
                        h = hpool.tile([128, 2048], bf16, tag="h")
                        relu(h[:, :], ps[:, :], l < 3)
                        st[s]["h"] = h

                if ok(t - 7):
                    s = t - 7
                    ps = pspool.tile([128, 2048], f32, tag="ps")
                    for hh in range(4):
                        nc.tensor.matmul(
                            ps[:, 512 * hh : 512 * hh + 512],
                            lhsT=wsl(6),
                            rhs=st[s]["h"][:, 512 * hh : 512 * hh + 512],
                            start=True,
                            stop=True,
                        )
                    ot = otpool.tile([128, 2048], f32, tag="ot")
                    nc.vector.transpose(ot[:, :], ps[:, :])
                    nc.sync.dma_start(o_r[s], ot[:, :])
    nc.compile()
    return nc


def prep_weights(input_weight, hidden_weights, output_weights):
    """Build the 7 block-diagonal 128x128 stationaries, concat to [128, 896]."""
    hid_filter = np.kron(np.eye(4, dtype=np.float32), np.ones((8, 8), np.float32))
    out_filter = np.kron(np.eye(8, dtype=np.float32), np.ones((4, 3), np.float32))
    whm = hid_filter[None] * np.asarray(hidden_weights, np.float32)  # [4,32,32]
    wom = out_filter * np.asarray(output_weights, np.float32)  # [32,24]
    w_in = np.asarray(input_weight, np.float32)  # [64,32]

    mats = []
    for fb in range(2):
        mats.append(np.kron(np.eye(4, dtype=np.float32), w_in[32 * fb : 32 * fb + 32]))
    for l in range(4):
        mats.append(np.kron(np.eye(4, dtype=np.float32), whm[l]))
    wo_pad = np.zeros((32, 32), np.float32)
    wo_pad[:, :24] = wom
    mats.append(np.kron(np.eye(4, dtype=np.float32), wo_pad))
    return np.concatenate(mats, axis=1)  # [128, 7*128]


def enable_ldw_opt():
    """Walrus can dedupe back-to-back LDWEIGHTS with identical stationaries;
    concourse pins --enable-ldw-opt=false. Flip it via the run_command seam."""
    from concourse import bass_utils as bu

    if getattr(bu, "_ldw_opt_patched", False):
        return
    orig = bu.run_command

    def patched(cmd, *a, **kw):
        if isinstance(cmd, list):
            cmd = [
                "--enable-ldw-opt=true" if c == "--enable-ldw-opt=false" else c
                for c in cmd
            ]
        return orig(cmd, *a, **kw)

    bu.run_command = patched
    bu._ldw_opt_patched = True


def to_bf16(a):
    import ml_dtypes

    return np.asarray(a, np.float32).astype(ml_dtypes.bfloat16)


def kernel(x, input_weight, hidden_weights, output_weights):
    from concourse.bass_utils import run_bass_kernel_spmd

    x = to_bf16(x)
    wbd = to_bf16(prep_weights(input_weight, hidden_weights, output_weights))

    nc = build_nc(R)
    shards = x.reshape(N_CORES, R, 64)
    in_maps = [{"x": shards[i], "wbd": wbd} for i in range(N_CORES)]
    res = run_bass_kernel_spmd(nc, in_maps, core_ids=list(range(N_CORES)))
    outs = [
        np.asarray(res.results[i]["out"]).astype(np.float32)[:, :24]
        for i in range(N_CORES)
    ]
    return np.concatenate(outs, axis=0)


# revision 40
# speedup vs baseline: 1.1114x; 1.1114x over previous
"""Trainium2 Bass kernel for nn_BD dense MLP (block-diagonal hidden layers).

Network: x[B,64] -> relu(x@W_in)[B,32] -> 4x relu(h@(mask*W_h))[B,32]
         -> h@(mask*W_out)[B,24]

Strategy (pure data parallel over 8 cores, B=1048576, R=131072 rows/core):
 - x loaded batch-major contiguously; DVE 32x32 block-transpose flips each
   [32 batch x 32 feat] block to feature-major. The resulting batch
   permutation is undone by the output DMA access pattern.
 - All matmuls run feature-major: weights stationary (lhsT), activations
   moving (rhs, N=512). 4 batch chunks sit at partition groups 0..3 and use
   diagonal PE tile positions (rows 32c, cols 32c) so 4 matmuls stream
   concurrently in different 32x32 sub-arrays.
 - ReLU fused into the PSUM->SBUF move on ScalarE/VectorE at full 128
   partitions.
 - Output written padded [R,32]; host strips to 24 cols.
"""

import sys

import numpy as np

if "/opt/trn_rl_repo" not in sys.path:
    sys.path.insert(0, "/opt/trn_rl_repo")

N_CORES = 8
B_FULL = 1048576
R = B_FULL // N_CORES  # rows per core
SLAB = 8192  # rows per pipeline iteration
F32 = None  # set after import


def build_nc(rows=R, act_split=(True, True, True, True, False)):
    """Build the single-core SPMD Bass graph.

    act_split[l]: True -> relu on ScalarE, False -> relu on VectorE.
    """
    import concourse.bass as bass  # noqa: F401
    import concourse.mybir as mybir
    from concourse import bacc, tile

    f32 = mybir.dt.float32
    bf16 = mybir.dt.bfloat16
    nc = bacc.Bacc(None)

    x_ext = nc.declare_dram_parameter("x", [rows, 64], bf16, isOutput=False)
    # 7 block-diagonal 128x128 stationaries: L1 fb0, L1 fb1, L2..L5, L6
    wbd_ext = nc.declare_dram_parameter("wbd", [128, 896], bf16, isOutput=False)
    out_ext = nc.declare_dram_parameter("out", [rows, 32], f32, isOutput=True)

    n_slabs = rows // SLAB
    # x row r = s*8192 + p*64 + n  (p = SBUF partition, n = 0..63)
    x_r = x_ext.rearrange("(s p n) f -> s p (n f)", p=128, n=64)
    # out row r = s*8192 + pg*2048 + b*64 + n ; partition = 32*pg + b
    o_r = out_ext.rearrange("(s pg b n) c -> s (pg b) (n c)", pg=4, b=32, n=64)

    Relu = mybir.ActivationFunctionType.Relu

    NILV = 4  # slabs processed in interleaved groups
    assert n_slabs % NILV == 0 or n_slabs < NILV

    with tile.TileContext(nc) as tc:
        with (
            tc.tile_pool(name="const", bufs=1) as cpool,
            tc.tile_pool(name="xin", bufs=4) as xpool,
            tc.tile_pool(name="xt", bufs=3) as xtpool,
            tc.tile_pool(name="h", bufs=12) as hpool,
            tc.tile_pool(name="ps", bufs=4, space="PSUM") as pspool,
            tc.tile_pool(name="ot", bufs=3) as otpool,
        ):
            wbd = cpool.tile([128, 896], bf16, tag="wbd")
            nc.sync.dma_start(wbd[:, :], wbd_ext[:, :])

            def wsl(i):
                return wbd[:, 128 * i : 128 * i + 128]

            def relu(out_t, in_t, on_act):
                if on_act:
                    nc.scalar.activation(out_t, in_t, Relu)
                else:
                    nc.vector.tensor_scalar_max(out_t, in_t, 0.0)

            # Fully skewed software pipeline: step t advances slab t-k
            # through stage k. Stages: 0 load, 1 xT, 2 L1+relu1,
            # 3..6 L2..L5+relu, 7 L6+oT+store.
            st = [dict() for _ in range(n_slabs)]

            def ok(i):
                return 0 <= i < n_slabs

            for t in range(n_slabs + 8):
                if ok(t):
                    x_sb = xpool.tile([128, 4096], bf16, tag="x")
                    nc.sync.dma_start(x_sb[:, :], x_r[t])
                    st[t]["x"] = x_sb

                if ok(t - 1):
                    s = t - 1
                    xt = xtpool.tile([128, 4096], bf16, tag="xt")
                    nc.vector.transpose(xt[:, :], st[s]["x"][:, :])
                    st[s]["xt"] = xt[:, :].rearrange(
                        "p (n fb b) -> p n fb b", fb=2, b=32
                    )

                if ok(t - 2):
                    s = t - 2
                    h = hpool.tile([128, 2048], bf16, tag="h")
                    for half in range(2):
                        ps = pspool.tile([128, 1024], f32, tag="ps")
                        for fb in range(2):
                            for hh in range(2):
                                q = 2 * half + hh
                                nc.tensor.matmul(
                                    ps[:, 512 * hh : 512 * hh + 512],
                                    lhsT=wsl(fb),
                                    rhs=st[s]["xt"][:, 16 * q : 16 * q + 16, fb, :],
                                    start=(fb == 0),
                                    stop=(fb == 1),
                                )
                        relu(
                            h[:, 1024 * half : 1024 * half + 1024], ps[:, :], True
                        )
                    st[s]["h"] = h

                for l in range(4):
                    s = t - 3 - l
                    if ok(s):
                        on_act = (l < 3) or (s % 2 == 0)
                        h = hpool.tile([128, 2048], bf16, tag="h")
                        for half in range(2):
                            ps = pspool.tile([128, 1024], f32, tag="ps")
                            for hh in range(2):
                                nc.tensor.matmul(
                                    ps[:, 512 * hh : 512 * hh + 512],
                                    lhsT=wsl(2 + l),
                                    rhs=st[s]["h"][
                                        :,
                                        1024 * half
                                        + 512 * hh : 1024 * half
                                        + 512 * hh
                                        + 512,
                                    ],
                                    start=True,
                                    stop=True,
                                )
                            relu(
                                h[:, 1024 * half : 1024 * half + 1024],
                                ps[:, :],
                                on_act,
                            )
                        st[s]["h"] = h

                if ok(t - 7):
                    s = t - 7
                    ot = otpool.tile([128, 2048], f32, tag="ot")
                    for half in range(2):
                        ps = pspool.tile([128, 1024], f32, tag="ps")
                        for hh in range(2):
                            nc.tensor.matmul(
                                ps[:, 512 * hh : 512 * hh + 512],
                                lhsT=wsl(6),
                                rhs=st[s]["h"][
                                    :,
                                    1024 * half
                                    + 512 * hh : 1024 * half
                                    + 512 * hh
                                    + 512,
                                ],
                                start=True,
                                stop=True,
                            )
                        nc.vector.transpose(
                            ot[:, 1024 * half : 1024 * half + 1024], ps[:, :]
                        )
                    nc.sync.dma_start(o_r[s], ot[:, :])
    nc.compile()
    return nc


def prep_weights(input_weight, hidden_weights, output_weights):
    """Build the 7 block-diagonal 128x128 stationaries, concat to [128, 896]."""
    hid_filter = np.kron(np.eye(4, dtype=np.float32), np.ones((8, 8), np.float32))
    out_filter = np.kron(np.eye(8, dtype=np.float32), np.ones((4, 3), np.float32))
    whm = hid_filter[None] * np.asarray(hidden_weights, np.float32)  # [4,32,32]
    wom = out_filter * np.asarray(output_weights, np.float32)  # [32,24]
    w_in = np.asarray(input_weight, np.float32)  # [64,32]

    mats = []
    for fb in range(2):
        mats.append(np.kron(np.eye(4, dtype=np.float32), w_in[32 * fb : 32 * fb + 32]))
    for l in range(4):
        mats.append(np.kron(np.eye(4, dtype=np.float32), whm[l]))
    wo_pad = np.zeros((32, 32), np.float32)
    wo_pad[:, :24] = wom
    mats.append(np.kron(np.eye(4, dtype=np.float32), wo_pad))
    return np.concatenate(mats, axis=1)  # [128, 7*128]


def enable_ldw_opt():
    """Walrus can dedupe back-to-back LDWEIGHTS with identical stationaries;
    concourse pins --enable-ldw-opt=false. Flip it via the run_command seam."""
    from concourse import bass_utils as bu

    if getattr(bu, "_ldw_opt_patched", False):
        return
    orig = bu.run_command

    def patched(cmd, *a, **kw):
        if isinstance(cmd, list):
            cmd = [
                "--enable-ldw-opt=true" if c == "--enable-ldw-opt=false" else c
                for c in cmd
            ]
        return orig(cmd, *a, **kw)

    bu.run_command = patched
    bu._ldw_opt_patched = True


def to_bf16(a):
    import ml_dtypes

    return np.asarray(a, np.float32).astype(ml_dtypes.bfloat16)


def kernel(x, input_weight, hidden_weights, output_weights):
    from concourse.bass_utils import run_bass_kernel_spmd

    x = to_bf16(x)
    wbd = to_bf16(prep_weights(input_weight, hidden_weights, output_weights))

    nc = build_nc(R)
    shards = x.reshape(N_CORES, R, 64)
    in_maps = [{"x": shards[i], "wbd": wbd} for i in range(N_CORES)]
    res = run_bass_kernel_spmd(nc, in_maps, core_ids=list(range(N_CORES)))
    outs = [
        np.asarray(res.results[i]["out"]).astype(np.float32)[:, :24]
        for i in range(N_CORES)
    ]
    return np.concatenate(outs, axis=0)


# revision 41
# speedup vs baseline: 1.2386x; 1.1145x over previous
"""Trainium2 Bass kernel for nn_BD dense MLP (block-diagonal hidden layers).

Network: x[B,64] -> relu(x@W_in)[B,32] -> 4x relu(h@(mask*W_h))[B,32]
         -> h@(mask*W_out)[B,24]

Strategy (pure data parallel over 8 cores, B=1048576, R=131072 rows/core):
 - x loaded batch-major contiguously; DVE 32x32 block-transpose flips each
   [32 batch x 32 feat] block to feature-major. The resulting batch
   permutation is undone by the output DMA access pattern.
 - All matmuls run feature-major: weights stationary (lhsT), activations
   moving (rhs, N=512). 4 batch chunks sit at partition groups 0..3 and use
   diagonal PE tile positions (rows 32c, cols 32c) so 4 matmuls stream
   concurrently in different 32x32 sub-arrays.
 - ReLU fused into the PSUM->SBUF move on ScalarE/VectorE at full 128
   partitions.
 - Output written padded [R,32]; host strips to 24 cols.
"""

import sys

import numpy as np

if "/opt/trn_rl_repo" not in sys.path:
    sys.path.insert(0, "/opt/trn_rl_repo")

N_CORES = 8
B_FULL = 1048576
R = B_FULL // N_CORES  # rows per core
SLAB = 4096  # rows per pipeline iteration
F32 = None  # set after import


def build_nc(rows=R, act_split=(True, True, True, True, False)):
    """Build the single-core SPMD Bass graph.

    act_split[l]: True -> relu on ScalarE, False -> relu on VectorE.
    """
    import concourse.bass as bass  # noqa: F401
    import concourse.mybir as mybir
    from concourse import bacc, tile

    f32 = mybir.dt.float32
    bf16 = mybir.dt.bfloat16
    nc = bacc.Bacc(None)

    x_ext = nc.declare_dram_parameter("x", [rows, 64], bf16, isOutput=False)
    # 7 block-diagonal 128x128 stationaries: L1 fb0, L1 fb1, L2..L5, L6
    wbd_ext = nc.declare_dram_parameter("wbd", [128, 896], bf16, isOutput=False)
    out_ext = nc.declare_dram_parameter("out", [rows, 32], f32, isOutput=True)

    n_slabs = rows // SLAB
    # x row r = s*4096 + p*32 + n  (p = SBUF partition, n = 0..31)
    x_r = x_ext.rearrange("(s p n) f -> s p (n f)", p=128, n=32)
    # out row r = s*4096 + pg*1024 + b*32 + n ; partition = 32*pg + b
    o_r = out_ext.rearrange("(s pg b n) c -> s (pg b) (n c)", pg=4, b=32, n=32)

    Relu = mybir.ActivationFunctionType.Relu

    NILV = 4  # slabs processed in interleaved groups
    assert n_slabs % NILV == 0 or n_slabs < NILV

    with tile.TileContext(nc) as tc:
        with (
            tc.tile_pool(name="const", bufs=1) as cpool,
            tc.tile_pool(name="xin", bufs=4) as xpool,
            tc.tile_pool(name="xt", bufs=3) as xtpool,
            tc.tile_pool(name="h", bufs=12) as hpool,
            tc.tile_pool(name="ps", bufs=4, space="PSUM") as pspool,
            tc.tile_pool(name="ot", bufs=3) as otpool,
        ):
            wbd = cpool.tile([128, 896], bf16, tag="wbd")
            nc.sync.dma_start(wbd[:, :], wbd_ext[:, :])

            def wsl(i):
                return wbd[:, 128 * i : 128 * i + 128]

            def relu(out_t, in_t, on_act):
                if on_act:
                    nc.scalar.activation(out_t, in_t, Relu)
                else:
                    nc.vector.tensor_scalar_max(out_t, in_t, 0.0)

            # Fully skewed software pipeline: step t advances slab t-k
            # through stage k. Stages: 0 load, 1 xT, 2 L1+relu1,
            # 3..6 L2..L5+relu, 7 L6+oT+store.
            st = [dict() for _ in range(n_slabs)]

            def ok(i):
                return 0 <= i < n_slabs

            for t in range(n_slabs + 8):
                if ok(t):
                    x_sb = xpool.tile([128, 2048], bf16, tag="x")
                    nc.sync.dma_start(x_sb[:, :], x_r[t])
                    st[t]["x"] = x_sb

                if ok(t - 1):
                    s = t - 1
                    xt = xtpool.tile([128, 2048], bf16, tag="xt")
                    nc.vector.transpose(xt[:, :], st[s]["x"][:, :])
                    st[s]["xt"] = xt[:, :].rearrange(
                        "p (n fb b) -> p n fb b", fb=2, b=32
                    )

                if ok(t - 2):
                    s = t - 2
                    ps = pspool.tile([128, 1024], f32, tag="ps")
                    for hh in range(2):
                        for fb in range(2):
                            nc.tensor.matmul(
                                ps[:, 512 * hh : 512 * hh + 512],
                                lhsT=wsl(fb),
                                rhs=st[s]["xt"][:, 16 * hh : 16 * hh + 16, fb, :],
                                start=(fb == 0),
                                stop=(fb == 1),
                            )
                    h = hpool.tile([128, 1024], bf16, tag="h")
                    relu(h[:, :], ps[:, :], True)
                    st[s]["h"] = h

                for l in range(4):
                    s = t - 3 - l
                    if ok(s):
                        on_act = (l < 3) or (s % 4 == 0)
                        ps = pspool.tile([128, 1024], f32, tag="ps")
                        for hh in range(2):
                            nc.tensor.matmul(
                                ps[:, 512 * hh : 512 * hh + 512],
                                lhsT=wsl(2 + l),
                                rhs=st[s]["h"][:, 512 * hh : 512 * hh + 512],
                                start=True,
                                stop=True,
                            )
                        h = hpool.tile([128, 1024], bf16, tag="h")
                        relu(h[:, :], ps[:, :], on_act)
                        st[s]["h"] = h

                if ok(t - 7):
                    s = t - 7
                    ps = pspool.tile([128, 1024], f32, tag="ps")
                    for hh in range(2):
                        nc.tensor.matmul(
                            ps[:, 512 * hh : 512 * hh + 512],
                            lhsT=wsl(6),
                            rhs=st[s]["h"][:, 512 * hh : 512 * hh + 512],
                            start=True,
                            stop=True,
                        )
                    ot = otpool.tile([128, 1024], f32, tag="ot")
                    nc.vector.transpose(ot[:, :], ps[:, :])
                    nc.sync.dma_start(o_r[s], ot[:, :])
    nc.compile()
    return nc


def prep_weights(input_weight, hidden_weights, output_weights):
    """Build the 7 block-diagonal 128x128 stationaries, concat to [128, 896]."""
    hid_filter = np.kron(np.eye(4, dtype=np.float32), np.ones((8, 8), np.float32))
    out_filter = np.kron(np.eye(8, dtype=np.float32), np.ones((4, 3), np.float32))
    whm = hid_filter[None] * np.asarray(hidden_weights, np.float32)  # [4,32,32]
    wom = out_filter * np.asarray(output_weights, np.float32)  # [32,24]
    w_in = np.asarray(input_weight, np.float32)  # [64,32]

    mats = []
    for fb in range(2):
        mats.append(np.kron(np.eye(4, dtype=np.float32), w_in[32 * fb : 32 * fb + 32]))
    for l in range(4):
        mats.append(np.kron(np.eye(4, dtype=np.float32), whm[l]))
    wo_pad = np.zeros((32, 32), np.float32)
    wo_pad[:, :24] = wom
    mats.append(np.kron(np.eye(4, dtype=np.float32), wo_pad))
    return np.concatenate(mats, axis=1)  # [128, 7*128]


def enable_ldw_opt():
    """Walrus can dedupe back-to-back LDWEIGHTS with identical stationaries;
    concourse pins --enable-ldw-opt=false. Flip it via the run_command seam."""
    from concourse import bass_utils as bu

    if getattr(bu, "_ldw_opt_patched", False):
        return
    orig = bu.run_command

    def patched(cmd, *a, **kw):
        if isinstance(cmd, list):
            cmd = [
                "--enable-ldw-opt=true" if c == "--enable-ldw-opt=false" else c
                for c in cmd
            ]
        return orig(cmd, *a, **kw)

    bu.run_command = patched
    bu._ldw_opt_patched = True


def to_bf16(a):
    import ml_dtypes

    return np.asarray(a, np.float32).astype(ml_dtypes.bfloat16)


def kernel(x, input_weight, hidden_weights, output_weights):
    from concourse.bass_utils import run_bass_kernel_spmd

    x = to_bf16(x)
    wbd = to_bf16(prep_weights(input_weight, hidden_weights, output_weights))

    nc = build_nc(R)
    shards = x.reshape(N_CORES, R, 64)
    in_maps = [{"x": shards[i], "wbd": wbd} for i in range(N_CORES)]
    res = run_bass_kernel_spmd(nc, in_maps, core_ids=list(range(N_CORES)))
    outs = [
        np.asarray(res.results[i]["out"]).astype(np.float32)[:, :24]
        for i in range(N_CORES)
    ]
    return np.concatenate(outs, axis=0)


# revision 42
# speedup vs baseline: 1.2687x; 1.0243x over previous
"""Trainium2 Bass kernel for nn_BD dense MLP (block-diagonal hidden layers).

Network: x[B,64] -> relu(x@W_in)[B,32] -> 4x relu(h@(mask*W_h))[B,32]
         -> h@(mask*W_out)[B,24]

Strategy (pure data parallel over 8 cores, B=1048576, R=131072 rows/core):
 - x loaded batch-major contiguously; DVE 32x32 block-transpose flips each
   [32 batch x 32 feat] block to feature-major. The resulting batch
   permutation is undone by the output DMA access pattern.
 - All matmuls run feature-major: weights stationary (lhsT), activations
   moving (rhs, N=512). 4 batch chunks sit at partition groups 0..3 and use
   diagonal PE tile positions (rows 32c, cols 32c) so 4 matmuls stream
   concurrently in different 32x32 sub-arrays.
 - ReLU fused into the PSUM->SBUF move on ScalarE/VectorE at full 128
   partitions.
 - Output written padded [R,32]; host strips to 24 cols.
"""

import sys

import numpy as np

if "/opt/trn_rl_repo" not in sys.path:
    sys.path.insert(0, "/opt/trn_rl_repo")

N_CORES = 8
B_FULL = 1048576
R = B_FULL // N_CORES  # rows per core
SLAB = 4096  # rows per pipeline iteration
F32 = None  # set after import


def build_nc(rows=R, act_split=(True, True, True, True, False)):
    """Build the single-core SPMD Bass graph.

    act_split[l]: True -> relu on ScalarE, False -> relu on VectorE.
    """
    import concourse.bass as bass  # noqa: F401
    import concourse.mybir as mybir
    from concourse import bacc, tile

    f32 = mybir.dt.float32
    bf16 = mybir.dt.bfloat16
    nc = bacc.Bacc(None)

    x_ext = nc.declare_dram_parameter("x", [rows, 64], bf16, isOutput=False)
    # 7 block-diagonal 128x128 stationaries: L1 fb0, L1 fb1, L2..L5, L6
    wbd_ext = nc.declare_dram_parameter("wbd", [128, 896], bf16, isOutput=False)
    out_ext = nc.declare_dram_parameter("out", [rows, 32], f32, isOutput=True)

    n_slabs = rows // SLAB
    # x row r = s*4096 + p*32 + n  (p = SBUF partition, n = 0..31)
    x_r = x_ext.rearrange("(s p n) f -> s p (n f)", p=128, n=32)
    # out row r = s*4096 + pg*1024 + b*32 + n ; partition = 32*pg + b
    o_r = out_ext.rearrange("(s pg b n) c -> s (pg b) (n c)", pg=4, b=32, n=32)

    Relu = mybir.ActivationFunctionType.Relu

    NILV = 4  # slabs processed in interleaved groups
    assert n_slabs % NILV == 0 or n_slabs < NILV

    with tile.TileContext(nc) as tc:
        with (
            tc.tile_pool(name="const", bufs=1) as cpool,
            tc.tile_pool(name="xin", bufs=4) as xpool,
            tc.tile_pool(name="xt", bufs=3) as xtpool,
            tc.tile_pool(name="h", bufs=12) as hpool,
            tc.tile_pool(name="ps", bufs=4, space="PSUM") as pspool,
            tc.tile_pool(name="ot", bufs=3) as otpool,
        ):
            wbd = cpool.tile([128, 896], bf16, tag="wbd")
            nc.sync.dma_start(wbd[:, :], wbd_ext[:, :])

            def wsl(i):
                return wbd[:, 128 * i : 128 * i + 128]

            def relu(out_t, in_t, on_act):
                if on_act:
                    nc.scalar.activation(out_t, in_t, Relu)
                else:
                    nc.vector.tensor_scalar_max(out_t, in_t, 0.0)

            # Fully skewed software pipeline: step t advances slab t-k
            # through stage k. Stages: 0 load, 1 xT, 2 L1+relu1,
            # 3..6 L2..L5+relu, 7 L6+oT+store.
            st = [dict() for _ in range(n_slabs)]

            def ok(i):
                return 0 <= i < n_slabs

            for t in range(n_slabs + 8):
                if ok(t):
                    x_sb = xpool.tile([128, 2048], bf16, tag="x")
                    nc.sync.dma_start(x_sb[:, :], x_r[t])
                    st[t]["x"] = x_sb

                if ok(t - 1):
                    s = t - 1
                    xt = xtpool.tile([128, 2048], bf16, tag="xt")
                    nc.vector.transpose(xt[:, :], st[s]["x"][:, :])
                    st[s]["xt"] = xt[:, :].rearrange(
                        "p (n fb b) -> p n fb b", fb=2, b=32
                    )

                if ok(t - 2):
                    s = t - 2
                    ps = pspool.tile([128, 1024], f32, tag="ps")
                    for hh in range(2):
                        for fb in range(2):
                            nc.tensor.matmul(
                                ps[:, 512 * hh : 512 * hh + 512],
                                lhsT=wsl(fb),
                                rhs=st[s]["xt"][:, 16 * hh : 16 * hh + 16, fb, :],
                                start=(fb == 0),
                                stop=(fb == 1),
                            )
                    h = hpool.tile([128, 1024], bf16, tag="h")
                    relu(h[:, :], ps[:, :], True)
                    st[s]["h"] = h

                for l in range(4):
                    s = t - 3 - l
                    if ok(s):
                        on_act = l < 3
                        ps = pspool.tile([128, 1024], f32, tag="ps")
                        for hh in range(2):
                            nc.tensor.matmul(
                                ps[:, 512 * hh : 512 * hh + 512],
                                lhsT=wsl(2 + l),
                                rhs=st[s]["h"][:, 512 * hh : 512 * hh + 512],
                                start=True,
                                stop=True,
                            )
                        h = hpool.tile([128, 1024], bf16, tag="h")
                        relu(h[:, :], ps[:, :], on_act)
                        st[s]["h"] = h

                if ok(t - 7):
                    s = t - 7
                    ps = pspool.tile([128, 1024], f32, tag="ps")
                    for hh in range(2):
                        nc.tensor.matmul(
                            ps[:, 512 * hh : 512 * hh + 512],
                            lhsT=wsl(6),
                            rhs=st[s]["h"][:, 512 * hh : 512 * hh + 512],
                            start=True,
                            stop=True,
                        )
                    ot = otpool.tile([128, 1024], f32, tag="ot")
                    nc.vector.transpose(ot[:, :], ps[:, :])
                    nc.sync.dma_start(o_r[s], ot[:, :])
    nc.compile()
    return nc


def prep_weights(input_weight, hidden_weights, output_weights):
    """Build the 7 block-diagonal 128x128 stationaries, concat to [128, 896]."""
    hid_filter = np.kron(np.eye(4, dtype=np.float32), np.ones((8, 8), np.float32))
    out_filter = np.kron(np.eye(8, dtype=np.float32), np.ones((4, 3), np.float32))
    whm = hid_filter[None] * np.asarray(hidden_weights, np.float32)  # [4,32,32]
    wom = out_filter * np.asarray(output_weights, np.float32)  # [32,24]
    w_in = np.asarray(input_weight, np.float32)  # [64,32]

    mats = []
    for fb in range(2):
        mats.append(np.kron(np.eye(4, dtype=np.float32), w_in[32 * fb : 32 * fb + 32]))
    for l in range(4):
        mats.append(np.kron(np.eye(4, dtype=np.float32), whm[l]))
    wo_pad = np.zeros((32, 32), np.float32)
    wo_pad[:, :24] = wom
    mats.append(np.kron(np.eye(4, dtype=np.float32), wo_pad))
    return np.concatenate(mats, axis=1)  # [128, 7*128]


def enable_ldw_opt():
    """Walrus can dedupe back-to-back LDWEIGHTS with identical stationaries;
    concourse pins --enable-ldw-opt=false. Flip it via the run_command seam."""
    from concourse import bass_utils as bu

    if getattr(bu, "_ldw_opt_patched", False):
        return
    orig = bu.run_command

    def patched(cmd, *a, **kw):
        if isinstance(cmd, list):
            cmd = [
                "--enable-ldw-opt=true" if c == "--enable-ldw-opt=false" else c
                for c in cmd
            ]
        return orig(cmd, *a, **kw)

    bu.run_command = patched
    bu._ldw_opt_patched = True


def to_bf16(a):
    import ml_dtypes

    return np.asarray(a, np.float32).astype(ml_dtypes.bfloat16)


def kernel(x, input_weight, hidden_weights, output_weights):
    from concourse.bass_utils import run_bass_kernel_spmd

    x = to_bf16(x)
    wbd = to_bf16(prep_weights(input_weight, hidden_weights, output_weights))

    nc = build_nc(R)
    shards = x.reshape(N_CORES, R, 64)
    in_maps = [{"x": shards[i], "wbd": wbd} for i in range(N_CORES)]
    res = run_bass_kernel_spmd(nc, in_maps, core_ids=list(range(N_CORES)))
    outs = [
        np.asarray(res.results[i]["out"]).astype(np.float32)[:, :24]
        for i in range(N_CORES)
    ]
    return np.concatenate(outs, axis=0)
